# revision 23
# baseline (speedup 1.0000x reference)
"""Multi-head causal attention (B=2, T=2048, H=16, D=64, C=1024) on 8 trn2 cores.

Sharding: tensor-parallel over heads. Each core owns 2 heads (both batches):
  - computes Q^T/K^T/V^T for its heads over all 4096 tokens
  - causal attention in transposed orientation (S^T[k,q]) so no P transpose
  - partial output projection outT_partial[c, t] = Wo_slice^T @ O^T
Host sums the 8 partials (the "all-reduce"), adds bias, transposes back.

Matmul operands are bf16 (fp32 PSUM accumulation); output written fp32.
"""

import sys

sys.path.insert(0, "/opt/trn_rl_repo")

import ml_dtypes
import numpy as np

import concourse.bacc as bacc
import concourse.mybir as mybir
import concourse.tile as tile
from concourse.bass_utils import run_bass_kernel_spmd

B, T, C = 2, 2048, 1024
H, D = 16, 64
NT = B * T  # 4096 flattened tokens
N_CORES = 8
HPC = H // N_CORES  # 2 heads per core
FPC = HPC * D  # 128 features per core
CT = C // 128  # 8 contraction tiles for projections
TBLK = 512  # token block
NTB = NT // TBLK  # 8 token blocks
QB = T // TBLK  # 4 query blocks per batch
KT = T // 128  # 16 key tiles per batch

F32 = mybir.dt.float32
BF16 = mybir.dt.bfloat16


def build_program():
    nc = bacc.Bacc("TRN2", target_bir_lowering=False, debug=False)

    xt_d = nc.declare_dram_parameter("xt", [C, NT], BF16, isOutput=False)
    wq_d = nc.declare_dram_parameter("wq", [C, FPC], BF16, isOutput=False)
    wk_d = nc.declare_dram_parameter("wk", [C, FPC], BF16, isOutput=False)
    wv_d = nc.declare_dram_parameter("wv", [C, FPC], BF16, isOutput=False)
    wo_d = nc.declare_dram_parameter("wo", [FPC, C], BF16, isOutput=False)
    out_d = nc.declare_dram_parameter("outT", [C, NT], F32, isOutput=True)

    with tile.TileContext(nc) as tc:
        with (
            tc.tile_pool(name="slabs", bufs=1) as slabs,
            tc.tile_pool(name="xtp", bufs=3) as xtp,
            tc.tile_pool(name="esp", bufs=3) as esp,
            tc.tile_pool(name="vtp", bufs=2) as vtp,
            tc.tile_pool(name="rinp", bufs=2) as rinp,
            tc.tile_pool(name="outp", bufs=2) as outp,
            tc.tile_pool(name="psA", bufs=2, space="PSUM") as psA,
            tc.tile_pool(name="psS", bufs=2, space="PSUM") as psS,
            tc.tile_pool(name="psO", bufs=2, space="PSUM") as psO,
        ):
            # ---- persistent slabs
            qT = slabs.tile([128, NT], BF16, tag="qT")  # [2h*64d, t]
            kT = slabs.tile([128, NT], BF16, tag="kT")
            # V natural layout: per ktile_global: [128k, (ones | V_h0 | V_h1 | ones)]
            # PV stationary h0 = [:, ktg, 0:2, :] = [ones|V_h0] -> rowsum rows 0:64, O 64:128
            #               h1 = [:, ktg, 2:4, :] = [V_h1|ones] -> O rows 0:64, rowsum 64:128
            vN = slabs.tile([128, NTB * 4, 4, 64], BF16, tag="vN")
            oN = slabs.tile([128, NT], BF16, tag="oN")  # normalized O^T
            wq_s = slabs.tile([128, CT, FPC], BF16, tag="wq")
            wk_s = slabs.tile([128, CT, FPC], BF16, tag="wk")
            wv_s = slabs.tile([128, CT, FPC], BF16, tag="wv")
            wo_s = slabs.tile([128, C], BF16, tag="wo")  # [f, c]
            mtri = slabs.tile([128, 128], BF16, tag="mtri")  # 1 if j>=k else 0
            ident = slabs.tile([128, 128], F32, tag="ident")

            # ---- constants
            from concourse.masks import make_identity
            make_identity(nc, ident[:])
            mtri_f = slabs.tile([128, 128], F32, tag="mtri_f")
            nc.gpsimd.memset(mtri_f[:], 1.0)
            # keep 1.0 where (j - k) >= 0 else 0.0
            nc.gpsimd.affine_select(
                out=mtri_f[:],
                in_=mtri_f[:],
                compare_op=mybir.AluOpType.is_ge,
                fill=0.0,
                base=0,
                pattern=[[1, 128]],
                channel_multiplier=-1,
            )
            nc.vector.tensor_copy(mtri[:], mtri_f[:])
            # ones columns of vN (constant for the whole run)
            ones_f = slabs.tile([128, 64], F32, tag="ones_f")
            nc.gpsimd.memset(ones_f[:], 1.0)
            for ktg in range(NTB * 4):
                nc.vector.tensor_copy(vN[:, ktg, 0, :], ones_f[:])
                nc.vector.tensor_copy(vN[:, ktg, 3, :], ones_f[:])

            # ---- weight loads
            nc.sync.dma_start(wq_s[:], wq_d.rearrange("(ct p) f -> p ct f", p=128))
            nc.sync.dma_start(wk_s[:], wk_d.rearrange("(ct p) f -> p ct f", p=128))
            nc.sync.dma_start(wv_s[:], wv_d.rearrange("(ct p) f -> p ct f", p=128))
            nc.sync.dma_start(wo_s[:], wo_d[:])

            # ---- per batch: QKV projections for its tokens, then attention
            def qkv_for_tb(tb):
                xt_t = xtp.tile([128, CT, TBLK], BF16, tag="xt")
                nc.sync.dma_start(
                    xt_t[:],
                    xt_d[:, tb * TBLK : (tb + 1) * TBLK].rearrange(
                        "(ct p) t -> p ct t", p=128
                    ),
                )
                for name, w_s, dstT in (("q", wq_s, qT), ("k", wk_s, kT), ("v", wv_s, None)):
                    ps = psS.tile([128, HPC, TBLK], F32, tag="sT", name=f"ps_{name}_{tb}")
                    for ct in range(CT):
                        nc.tensor.matmul(
                            ps[:, 0, :],
                            w_s[:, ct, :],
                            xt_t[:, ct, :],
                            start=(ct == 0),
                            stop=(ct == CT - 1),
                        )
                    if dstT is not None:
                        nc.vector.tensor_copy(
                            dstT[:, tb * TBLK : (tb + 1) * TBLK], ps[:, 0, :]
                        )
                    else:
                        vt_t = vtp.tile([128, TBLK], F32, tag="vt")
                        nc.vector.tensor_copy(vt_t[:], ps[:, 0, :])
                        # transpose [64d, 128k] -> [128k, 64d] per head on PE
                        for sub in range(TBLK // 128):
                            ktg = tb * 4 + sub
                            for h in range(HPC):
                                tps = psA.tile([128, 64], F32, tag="ps")
                                nc.tensor.transpose(
                                    tps[:],
                                    vt_t[
                                        h * 64 : (h + 1) * 64,
                                        sub * 128 : (sub + 1) * 128,
                                    ],
                                    ident[h * 64 : (h + 1) * 64, h * 64 : (h + 1) * 64],
                                )
                                nc.vector.tensor_copy(vN[:, ktg, 1 + h, :], tps[:])

            # ---- attention per (batch, qblock), heads interleaved
            for tb in range(NTB):
                qkv_for_tb(tb)
            for b in range(B):
                for qb in range(QB):
                    t0 = b * T + qb * TBLK  # global token offset of this q block
                    O_ps = [
                        psO.tile([128, TBLK], F32, tag="O", name=f"O_{b}_{qb}_{h}")
                        for h in range(HPC)
                    ]
                    nkt = (qb + 1) * 4
                    for kt in range(nkt):
                        s = kt * 128 - qb * TBLK  # diag offset, >=0 on band
                        col0 = max(s, 0)
                        ktg = b * KT + kt
                        sT = psS.tile([128, HPC, TBLK], F32, tag="sT")
                        es = esp.tile([128, HPC, TBLK], BF16, tag="es")
                        for h in range(HPC):
                            hp = h * 64
                            nc.tensor.matmul(
                                sT[:, h, col0:TBLK],
                                kT[hp : hp + 64, b * T + kt * 128 : b * T + (kt + 1) * 128],
                                qT[hp : hp + 64, t0 + col0 : t0 + TBLK],
                                start=True,
                                stop=True,
                            )
                        nc.scalar.activation(
                            es[:, :, col0:TBLK],
                            sT[:, :, col0:TBLK],
                            mybir.ActivationFunctionType.Exp,
                            scale=0.125,
                        )
                        if s >= 0:  # diagonal tile: mask strict-lower triangle
                            for h in range(HPC):
                                nc.vector.tensor_mul(
                                    es[:, h, col0 : col0 + 128],
                                    es[:, h, col0 : col0 + 128],
                                    mtri[:],
                                )
                        for h in range(HPC):
                            vsta = vN[:, ktg, 0:2, :] if h == 0 else vN[:, ktg, 2:4, :]
                            nc.tensor.matmul(
                                O_ps[h][:, col0:TBLK],
                                vsta,
                                es[:, h, col0:TBLK],
                                start=(kt == 0),
                                stop=(kt == nkt - 1),
                            )
                    # normalize: O / rowsum (rowsum rows: h0 -> 64:128, h1 -> 0:64)
                    # approx recip needs SBUF input at partition base 0
                    rs = rinp.tile([64, TBLK], F32, tag="rs")
                    rin = rinp.tile([64, TBLK], F32, tag="rin")
                    nc.vector.tensor_copy(rs[0:64, :], O_ps[0][0:64, :])
                    nc.vector.reciprocal_approx_fast(rin[0:64, :], rs[0:64, :])
                    nc.vector.tensor_mul(
                        oN[0:64, t0 : t0 + TBLK], O_ps[0][64:128, :], rin[0:64, :]
                    )
                    rs2 = rinp.tile([64, TBLK], F32, tag="rs2")
                    rin2 = rinp.tile([64, TBLK], F32, tag="rin2")
                    nc.vector.tensor_copy(rs2[0:64, :], O_ps[1][64:128, :])
                    nc.vector.reciprocal_approx_fast(rin2[0:64, :], rs2[0:64, :])
                    nc.vector.tensor_mul(
                        oN[64:128, t0 : t0 + TBLK], O_ps[1][0:64, :], rin2[0:64, :]
                    )
                    # out-projection for this token block
                    ot = outp.tile([128, CT, TBLK], F32, tag="ot")
                    for ct in range(CT):
                        opst = psA.tile([128, TBLK], F32, tag="ps")
                        ops = opst[:]
                        nc.tensor.matmul(
                            ops,
                            wo_s[:, ct * 128 : (ct + 1) * 128],
                            oN[:, t0 : t0 + TBLK],
                            start=True,
                            stop=True,
                        )
                        if ct % 2 == 0:
                            nc.vector.tensor_copy(ot[:, ct, :], ops)
                        else:
                            nc.scalar.copy(ot[:, ct, :], ops)
                    nc.sync.dma_start(
                        out_d[:, t0 : t0 + TBLK].rearrange("(ct p) t -> p ct t", p=128),
                        ot[:],
                    )

    nc.compile()
    return nc


_NC_CACHE = None


def get_program():
    global _NC_CACHE
    if _NC_CACHE is None:
        _NC_CACHE = build_program()
    return _NC_CACHE


def make_in_maps(x, Wq, Wk, Wv, Wo):
    bf = ml_dtypes.bfloat16
    xt = np.ascontiguousarray(np.asarray(x, np.float32).reshape(NT, C).T).astype(bf)
    wq_b = np.asarray(Wq, np.float32).astype(bf)
    wk_b = np.asarray(Wk, np.float32).astype(bf)
    wv_b = np.asarray(Wv, np.float32).astype(bf)
    wo_b = np.asarray(Wo, np.float32).astype(bf)
    in_maps = []
    for cid in range(N_CORES):
        sl = slice(cid * FPC, (cid + 1) * FPC)
        in_maps.append(
            {
                "xt": xt,
                "wq": np.ascontiguousarray(wq_b[:, sl]),
                "wk": np.ascontiguousarray(wk_b[:, sl]),
                "wv": np.ascontiguousarray(wv_b[:, sl]),
                "wo": np.ascontiguousarray(wo_b[sl, :]),
            }
        )
    return in_maps


def kernel(x, Wq, Wk, Wv, Wo, bo, _trace=False, _tmpdir=None):
    x = np.asarray(x, dtype=np.float32)
    in_maps = make_in_maps(x, Wq, Wk, Wv, Wo)
    nc = get_program()
    res = run_bass_kernel_spmd(
        nc, in_maps, core_ids=list(range(N_CORES)), trace=_trace, tmpdir=_tmpdir
    )
    acc = res.results[0]["outT"].astype(np.float32)
    for i in range(1, N_CORES):
        acc = acc + res.results[i]["outT"]
    out = acc.T + np.asarray(bo, np.float32)[None, :]
    if _trace:
        kernel._last_results = res
    return out.reshape(B, T, C).astype(np.float32)


# revision 24
# speedup vs baseline: 1.0955x; 1.0955x over previous
"""Multi-head causal attention (B=2, T=2048, H=16, D=64, C=1024) on 8 trn2 cores.

Sharding: tensor-parallel over heads. Each core owns 2 heads (both batches):
  - computes Q^T/K^T/V^T for its heads over all 4096 tokens
  - causal attention in transposed orientation (S^T[k,q]) so no P transpose
  - partial output projection outT_partial[c, t] = Wo_slice^T @ O^T
Host sums the 8 partials (the "all-reduce"), adds bias, transposes back.

Matmul operands are bf16 (fp32 PSUM accumulation); output written fp32.
"""

import sys

sys.path.insert(0, "/opt/trn_rl_repo")

import ml_dtypes
import numpy as np

import concourse.bacc as bacc
import concourse.mybir as mybir
import concourse.tile as tile
from concourse.bass_utils import run_bass_kernel_spmd

B, T, C = 2, 2048, 1024
H, D = 16, 64
NT = B * T  # 4096 flattened tokens
N_CORES = 8
HPC = H // N_CORES  # 2 heads per core
FPC = HPC * D  # 128 features per core
CT = C // 128  # 8 contraction tiles for projections
TBLK = 512  # token block
NTB = NT // TBLK  # 8 token blocks
QB = T // TBLK  # 4 query blocks per batch
KT = T // 128  # 16 key tiles per batch

F32 = mybir.dt.float32
BF16 = mybir.dt.bfloat16


def build_program():
    nc = bacc.Bacc("TRN2", target_bir_lowering=False, debug=False)

    xt_d = nc.declare_dram_parameter("xt", [C, NT], BF16, isOutput=False)
    wq_d = nc.declare_dram_parameter("wq", [C, FPC], BF16, isOutput=False)
    wk_d = nc.declare_dram_parameter("wk", [C, FPC], BF16, isOutput=False)
    wv_d = nc.declare_dram_parameter("wv", [C, FPC], BF16, isOutput=False)
    wo_d = nc.declare_dram_parameter("wo", [FPC, C], BF16, isOutput=False)
    out_d = nc.declare_dram_parameter("outT", [C, NT], F32, isOutput=True)

    with tile.TileContext(nc) as tc:
        with (
            tc.tile_pool(name="slabs", bufs=1) as slabs,
            tc.tile_pool(name="xtp", bufs=3) as xtp,
            tc.tile_pool(name="esp", bufs=3) as esp,
            tc.tile_pool(name="vtp", bufs=2) as vtp,
            tc.tile_pool(name="rinp", bufs=2) as rinp,
            tc.tile_pool(name="outp", bufs=2) as outp,
            tc.tile_pool(name="psA", bufs=2, space="PSUM") as psA,
            tc.tile_pool(name="psS", bufs=2, space="PSUM") as psS,
            tc.tile_pool(name="psO", bufs=2, space="PSUM") as psO,
        ):
            # ---- persistent slabs
            qT = slabs.tile([128, NT], BF16, tag="qT")  # [2h*64d, t]
            kT = slabs.tile([128, NT], BF16, tag="kT")
            # V natural layout: per ktile_global: [128k, (ones | V_h0 | V_h1 | ones)]
            # PV stationary h0 = [:, ktg, 0:2, :] = [ones|V_h0] -> rowsum rows 0:64, O 64:128
            #               h1 = [:, ktg, 2:4, :] = [V_h1|ones] -> O rows 0:64, rowsum 64:128
            vN = slabs.tile([128, NTB * 4, 4, 64], BF16, tag="vN")
            oN = slabs.tile([128, NT], BF16, tag="oN")  # normalized O^T
            wq_s = slabs.tile([128, CT, FPC], BF16, tag="wq")
            wk_s = slabs.tile([128, CT, FPC], BF16, tag="wk")
            wv_s = slabs.tile([128, CT, FPC], BF16, tag="wv")
            wo_s = slabs.tile([128, C], BF16, tag="wo")  # [f, c]
            mtri = slabs.tile([128, 128], BF16, tag="mtri")  # 1 if j>=k else 0
            ident = slabs.tile([128, 128], F32, tag="ident")

            # ---- constants
            from concourse.masks import make_identity
            make_identity(nc, ident[:])
            mtri_f = slabs.tile([128, 128], F32, tag="mtri_f")
            nc.gpsimd.memset(mtri_f[:], 1.0)
            # keep 1.0 where (j - k) >= 0 else 0.0
            nc.gpsimd.affine_select(
                out=mtri_f[:],
                in_=mtri_f[:],
                compare_op=mybir.AluOpType.is_ge,
                fill=0.0,
                base=0,
                pattern=[[1, 128]],
                channel_multiplier=-1,
            )
            nc.vector.tensor_copy(mtri[:], mtri_f[:])
            # ones columns of vN (constant for the whole run)
            nc.gpsimd.memset(vN[:, :, 0, :], 1.0)
            nc.gpsimd.memset(vN[:, :, 3, :], 1.0)

            # ---- weight loads
            nc.sync.dma_start(wq_s[:], wq_d.rearrange("(ct p) f -> p ct f", p=128))
            nc.sync.dma_start(wk_s[:], wk_d.rearrange("(ct p) f -> p ct f", p=128))
            nc.sync.dma_start(wv_s[:], wv_d.rearrange("(ct p) f -> p ct f", p=128))
            nc.sync.dma_start(wo_s[:], wo_d[:])

            # ---- per batch: QKV projections for its tokens, then attention
            def qkv_for_tb(tb):
                xt_t = xtp.tile([128, CT, TBLK], BF16, tag="xt")
                nc.sync.dma_start(
                    xt_t[:],
                    xt_d[:, tb * TBLK : (tb + 1) * TBLK].rearrange(
                        "(ct p) t -> p ct t", p=128
                    ),
                )
                for name, w_s, dstT in (("q", wq_s, qT), ("k", wk_s, kT), ("v", wv_s, None)):
                    ps = psS.tile([128, HPC, TBLK], F32, tag="sT", name=f"ps_{name}_{tb}")
                    for ct in range(CT):
                        nc.tensor.matmul(
                            ps[:, 0, :],
                            w_s[:, ct, :],
                            xt_t[:, ct, :],
                            start=(ct == 0),
                            stop=(ct == CT - 1),
                        )
                    if dstT is not None:
                        nc.vector.tensor_copy(
                            dstT[:, tb * TBLK : (tb + 1) * TBLK], ps[:, 0, :]
                        )
                    else:
                        vt_t = vtp.tile([128, TBLK], F32, tag="vt")
                        nc.vector.tensor_copy(vt_t[:], ps[:, 0, :])
                        # transpose [128(d0|d1), 128k] -> [128k, (V_h0|V_h1)] on PE
                        for sub in range(TBLK // 128):
                            ktg = tb * 4 + sub
                            tps = psA.tile([128, 128], F32, tag="ps")
                            nc.tensor.transpose(
                                tps[:],
                                vt_t[:, sub * 128 : (sub + 1) * 128],
                                ident[:],
                            )
                            nc.vector.tensor_copy(
                                vN[:, ktg, 1:3, :].rearrange("p a c -> p (a c)"),
                                tps[:],
                            )

            # ---- attention per (batch, qblock), heads interleaved
            for tb in range(NTB):
                qkv_for_tb(tb)
            for b in range(B):
                for qb in range(QB):
                    t0 = b * T + qb * TBLK  # global token offset of this q block
                    O_ps = [
                        psO.tile([128, TBLK], F32, tag="O", name=f"O_{b}_{qb}_{h}")
                        for h in range(HPC)
                    ]
                    nkt = (qb + 1) * 4
                    for kt in range(nkt):
                        s = kt * 128 - qb * TBLK  # diag offset, >=0 on band
                        col0 = max(s, 0)
                        ktg = b * KT + kt
                        sT = psS.tile([128, HPC, TBLK], F32, tag="sT")
                        es = esp.tile([128, HPC, TBLK], BF16, tag="es")
                        for h in range(HPC):
                            hp = h * 64
                            nc.tensor.matmul(
                                sT[:, h, col0:TBLK],
                                kT[hp : hp + 64, b * T + kt * 128 : b * T + (kt + 1) * 128],
                                qT[hp : hp + 64, t0 + col0 : t0 + TBLK],
                                start=True,
                                stop=True,
                            )
                        nc.scalar.activation(
                            es[:, :, col0:TBLK],
                            sT[:, :, col0:TBLK],
                            mybir.ActivationFunctionType.Exp,
                            scale=0.125,
                        )
                        if s >= 0:  # diagonal tile: mask strict-lower triangle
                            for h in range(HPC):
                                nc.vector.tensor_mul(
                                    es[:, h, col0 : col0 + 128],
                                    es[:, h, col0 : col0 + 128],
                                    mtri[:],
                                )
                        for h in range(HPC):
                            vsta = vN[:, ktg, 0:2, :] if h == 0 else vN[:, ktg, 2:4, :]
                            nc.tensor.matmul(
                                O_ps[h][:, col0:TBLK],
                                vsta,
                                es[:, h, col0:TBLK],
                                start=(kt == 0),
                                stop=(kt == nkt - 1),
                            )
                    # normalize: O / rowsum (rowsum rows: h0 -> 64:128, h1 -> 0:64)
                    # approx recip needs SBUF input at partition base 0
                    rs = rinp.tile([64, TBLK], F32, tag="rs")
                    rin = rinp.tile([64, TBLK], F32, tag="rin")
                    nc.vector.tensor_copy(rs[0:64, :], O_ps[0][0:64, :])
                    nc.vector.reciprocal_approx_fast(rin[0:64, :], rs[0:64, :])
                    nc.vector.tensor_mul(
                        oN[0:64, t0 : t0 + TBLK], O_ps[0][64:128, :], rin[0:64, :]
                    )
                    rs2 = rinp.tile([64, TBLK], F32, tag="rs2")
                    rin2 = rinp.tile([64, TBLK], F32, tag="rin2")
                    nc.vector.tensor_copy(rs2[0:64, :], O_ps[1][64:128, :])
                    nc.vector.reciprocal_approx_fast(rin2[0:64, :], rs2[0:64, :])
                    nc.vector.tensor_mul(
                        oN[64:128, t0 : t0 + TBLK], O_ps[1][0:64, :], rin2[0:64, :]
                    )
                    # out-projection for this token block
                    ot = outp.tile([128, CT, TBLK], F32, tag="ot")
                    for ct in range(CT):
                        opst = psA.tile([128, TBLK], F32, tag="ps")
                        ops = opst[:]
                        nc.tensor.matmul(
                            ops,
                            wo_s[:, ct * 128 : (ct + 1) * 128],
                            oN[:, t0 : t0 + TBLK],
                            start=True,
                            stop=True,
                        )
                        if ct % 2 == 0:
                            nc.vector.tensor_copy(ot[:, ct, :], ops)
                        else:
                            nc.scalar.copy(ot[:, ct, :], ops)
                    nc.sync.dma_start(
                        out_d[:, t0 : t0 + TBLK].rearrange("(ct p) t -> p ct t", p=128),
                        ot[:],
                    )

    nc.compile()
    return nc


_NC_CACHE = None


def get_program():
    global _NC_CACHE
    if _NC_CACHE is None:
        _NC_CACHE = build_program()
    return _NC_CACHE


def make_in_maps(x, Wq, Wk, Wv, Wo):
    bf = ml_dtypes.bfloat16
    xt = np.ascontiguousarray(np.asarray(x, np.float32).reshape(NT, C).T).astype(bf)
    wq_b = np.asarray(Wq, np.float32).astype(bf)
    wk_b = np.asarray(Wk, np.float32).astype(bf)
    wv_b = np.asarray(Wv, np.float32).astype(bf)
    wo_b = np.asarray(Wo, np.float32).astype(bf)
    in_maps = []
    for cid in range(N_CORES):
        sl = slice(cid * FPC, (cid + 1) * FPC)
        in_maps.append(
            {
                "xt": xt,
                "wq": np.ascontiguousarray(wq_b[:, sl]),
                "wk": np.ascontiguousarray(wk_b[:, sl]),
                "wv": np.ascontiguousarray(wv_b[:, sl]),
                "wo": np.ascontiguousarray(wo_b[sl, :]),
            }
        )
    return in_maps


def kernel(x, Wq, Wk, Wv, Wo, bo, _trace=False, _tmpdir=None):
    x = np.asarray(x, dtype=np.float32)
    in_maps = make_in_maps(x, Wq, Wk, Wv, Wo)
    nc = get_program()
    res = run_bass_kernel_spmd(
        nc, in_maps, core_ids=list(range(N_CORES)), trace=_trace, tmpdir=_tmpdir
    )
    acc = res.results[0]["outT"].astype(np.float32)
    for i in range(1, N_CORES):
        acc = acc + res.results[i]["outT"]
    out = acc.T + np.asarray(bo, np.float32)[None, :]
    if _trace:
        kernel._last_results = res
    return out.reshape(B, T, C).astype(np.float32)
